# revision 6
# baseline (speedup 1.0000x reference)
"""ChittaEncoder retrieval-knn kernel for 8 trn2 NeuronCores.

Tensor-parallel retrieval, seed-sharded (4096 seeds/core):
  - q projection row-sharded, qT all-gathered (bf16 hi/lo split pair).
  - scores for all 2048 rows vs local seed shard via bf16 3-term split
    matmuls (qh*sh + qh*sl + ql*sh) accumulated in fp32 PSUM ->
    fp32-grade score precision at bf16 matmul speed.
  - exact local top-32 per row per seed-quarter via 4 rounds of DVE
    max8 + match_replace; allgather candidates; replicated global
    merge -> sorted global top-32 -> softmax -> attn.
  - field: w = [s >= t32] * exp(s - m - ln denom) bf16 mask-weights;
    partial field = w @ seeds_shard (bf16); ReduceScatter(add).
"""

import numpy as np

import concourse.bacc as bacc
import concourse.mybir as mybir
from concourse.tile import TileContext
from concourse.bass_utils import run_bass_kernel_spmd
from concourse.kernels.tile_matmul import make_identity

P = 128
B, D, N, K = 2048, 2048, 32768, 32
NCORE = 8
NSH = N // NCORE            # 4096 seeds per core
NHALF = 4                   # seed sub-shards per core (SBUF residency)
SH = NSH // NHALF           # 1024 seeds per sub-shard
RT = B // P                 # 16 row tiles
DCH = D // P                # 16 contraction chunks
BSH = B // NCORE            # 256 rows per core for q
ST = SH // 512              # seed tiles of 512 per sub-shard
FDT = D // 512              # d-tiles for the field matmul
CK = NHALF * K              # local candidates per row (128)
RSCALE = 1.0 / float(np.sqrt(np.float32(D)))

f32 = mybir.dt.float32
bf16 = mybir.dt.bfloat16
AF = mybir.ActivationFunctionType
ALU = mybir.AluOpType
AX = mybir.AxisListType

NEG_BIG = -1.0e30


def build():
    import concourse.bass_isa as bass_isa

    nc = bacc.Bacc()

    x_sh = nc.declare_dram_parameter("x_sh", [BSH, D], f32, isOutput=False)
    w_q = nc.declare_dram_parameter("w_q", [D, D], f32, isOutput=False)
    seeds_sh = nc.declare_dram_parameter("seeds_sh", [NSH, D], f32, isOutput=False)
    karma = nc.declare_dram_parameter("karma", [N], f32, isOutput=False)
    karma_sh = nc.declare_dram_parameter("karma_sh", [NSH], f32, isOutput=False)

    attn_out = nc.declare_dram_parameter("attn_out", [B, K], f32, isOutput=True)
    field_out = nc.declare_dram_parameter("field_out", [BSH, D], f32, isOutput=True)

    qpart_h = nc.dram_tensor("qpart_h", [D, BSH], bf16)
    qpart_l = nc.dram_tensor("qpart_l", [D, BSH], bf16)
    qg_h = nc.dram_tensor("qg_h", [NCORE, D, BSH], bf16, addr_space="Shared")
    qg_l = nc.dram_tensor("qg_l", [NCORE, D, BSH], bf16, addr_space="Shared")
    sh_dram = nc.dram_tensor("sh_dram", [NSH, D], bf16)
    scores_dram = nc.dram_tensor("scores_dram", [B, NSH], f32)
    cand_dram = nc.dram_tensor("cand_dram", [B, CK], f32)
    cand_g = nc.dram_tensor("cand_g", [NCORE, B, CK], f32, addr_space="Shared")
    fldp_dram = nc.dram_tensor("fldp_dram", [B, D], f32)
    rs_out = nc.dram_tensor("rs_out", [BSH, D], f32)

    with TileContext(nc) as tc:
        with tc.tile_pool(name="const", bufs=1) as const_pool:
            ident = const_pool.tile([P, P], bf16)
            make_identity(nc, ident)
            ones1 = const_pool.tile([1, P], f32)
            nc.vector.memset(ones1[:], 1.0)

            # =========== phase A: karma bias, q projection, scores, local topk =========
            with tc.tile_pool(name="mid", bufs=1) as mid_pool:
                bias_rep = mid_pool.tile([P, NSH], f32, tag="bias_rep")
                loc_all = mid_pool.tile([P, RT * CK], f32, tag="loc_all")

                # ---------------- karma bias ----------------
                with (
                    tc.tile_pool(name="kb", bufs=1) as kb_pool,
                    tc.tile_pool(name="kbp", bufs=2, space="PSUM") as kbp,
                ):
                    kt = kb_pool.tile([P, N // P], f32)
                    nc.sync.dma_start(out=kt[:], in_=karma[:].rearrange("(p f) -> p f", p=P))
                    kmax = kb_pool.tile([P, 1], f32)
                    nc.vector.tensor_reduce(kmax[:], kt[:], axis=AX.X, op=ALU.max)
                    gmax = kb_pool.tile([P, 1], f32)
                    nc.gpsimd.partition_all_reduce(gmax[:], kmax[:], channels=P,
                                                   reduce_op=bass_isa.ReduceOp.max)
                    ngmax = kb_pool.tile([P, 1], f32)
                    nc.vector.tensor_scalar_mul(ngmax[:], gmax[:], -1.0)
                    esc = kb_pool.tile([P, N // P], f32)
                    ssum = kb_pool.tile([P, 1], f32)
                    nc.scalar.activation(esc[:], kt[:], AF.Exp, bias=ngmax[:],
                                         accum_out=ssum[:])
                    stot = kb_pool.tile([P, 1], f32)
                    nc.gpsimd.partition_all_reduce(stot[:], ssum[:], channels=P,
                                                   reduce_op=bass_isa.ReduceOp.add)
                    lnt = kb_pool.tile([P, 1], f32)
                    nc.scalar.activation(lnt[:], stot[:], AF.Ln)
                    lse = kb_pool.tile([P, 1], f32)
                    nc.vector.tensor_add(lse[:], lnt[:], gmax[:])

                    ksh = kb_pool.tile([1, NSH], f32)
                    nc.sync.dma_start(out=ksh[:],
                                      in_=karma_sh[:].rearrange("(o f) -> o f", o=1))
                    bias_row = kb_pool.tile([1, NSH], f32)
                    nc.vector.tensor_scalar(bias_row[:], ksh[:], lse[0:1, 0:1], None,
                                            op0=ALU.subtract)
                    nc.vector.tensor_scalar_max(bias_row[:], bias_row[:], -10.0)
                    for j in range(NSH // 512):
                        ps = kbp.tile([P, 512], f32, tag="ps")
                        nc.tensor.matmul(ps[:], ones1[:],
                                         bias_row[:, j * 512:(j + 1) * 512],
                                         start=True, stop=True)
                        nc.vector.tensor_copy(bias_rep[:, j * 512:(j + 1) * 512], ps[:])

                # ---------------- q projection ----------------
                with (
                    tc.tile_pool(name="qp", bufs=2) as qp_pool,
                    tc.tile_pool(name="qx", bufs=1) as qx_pool,
                    tc.tile_pool(name="qwj", bufs=2) as qwj_pool,
                    tc.tile_pool(name="qps", bufs=2, space="PSUM") as qps_pool,
                    tc.tile_pool(name="qacc_ps", bufs=2, space="PSUM") as qacc_pool,
                ):
                    xh = qx_pool.tile([P, DCH * BSH], bf16, tag="xh")
                    xl = qx_pool.tile([P, DCH * BSH], bf16, tag="xl")
                    for i in range(BSH // P):
                        xrow = qp_pool.tile([P, D], f32, tag="xrow")
                        nc.sync.dma_start(out=xrow[:], in_=x_sh[i * P:(i + 1) * P, :])
                        xrow_b = qp_pool.tile([P, D], bf16, tag="xrow_b")
                        nc.vector.tensor_copy(xrow_b[:], xrow[:])
                        xrow_r = qp_pool.tile([P, D], bf16, tag="xrow_r")
                        nc.vector.scalar_tensor_tensor(xrow_r[:], xrow_b[:], -1.0, xrow[:],
                                                       op0=ALU.mult, op1=ALU.add)
                        for c in range(DCH):
                            psh = qps_pool.tile([P, P], bf16, tag="pst")
                            nc.tensor.transpose(psh[:], xrow_b[:, c * P:(c + 1) * P],
                                                ident[:])
                            nc.scalar.activation(
                                xh[:, c * BSH + i * P: c * BSH + (i + 1) * P],
                                psh[:], AF.Copy)
                            psl = qps_pool.tile([P, P], bf16, tag="pst")
                            nc.tensor.transpose(psl[:], xrow_r[:, c * P:(c + 1) * P],
                                                ident[:])
                            nc.scalar.activation(
                                xl[:, c * BSH + i * P: c * BSH + (i + 1) * P],
                                psl[:], AF.Copy)

                    for j in range(D // P):
                        wrow = qp_pool.tile([P, D], f32, tag="wrow")
                        nc.sync.dma_start(out=wrow[:], in_=w_q[j * P:(j + 1) * P, :])
                        wrow_b = qp_pool.tile([P, D], bf16, tag="wrow_b")
                        nc.vector.tensor_copy(wrow_b[:], wrow[:])
                        wrow_r = qp_pool.tile([P, D], bf16, tag="wrow_r")
                        nc.vector.scalar_tensor_tensor(wrow_r[:], wrow_b[:], -1.0, wrow[:],
                                                       op0=ALU.mult, op1=ALU.add)
                        wjh = qwj_pool.tile([P, DCH * P], bf16, tag="wjh")
                        wjl = qwj_pool.tile([P, DCH * P], bf16, tag="wjl")
                        for c in range(DCH):
                            psh = qps_pool.tile([P, P], bf16, tag="pst")
                            nc.tensor.transpose(psh[:], wrow_b[:, c * P:(c + 1) * P],
                                                ident[:])
                            nc.scalar.activation(wjh[:, c * P:(c + 1) * P], psh[:], AF.Copy)
                            psl = qps_pool.tile([P, P], bf16, tag="pst")
                            nc.tensor.transpose(psl[:], wrow_r[:, c * P:(c + 1) * P],
                                                ident[:])
                            nc.scalar.activation(wjl[:, c * P:(c + 1) * P], psl[:], AF.Copy)

                        ps = qacc_pool.tile([P, BSH], f32, tag="qacc")
                        first = True
                        for c in range(DCH):
                            lh = wjh[:, c * P:(c + 1) * P]
                            ll = wjl[:, c * P:(c + 1) * P]
                            rh = xh[:, c * BSH:(c + 1) * BSH]
                            rl = xl[:, c * BSH:(c + 1) * BSH]
                            nc.tensor.matmul(ps[:], lh, rh, start=first, stop=False)
                            first = False
                            nc.tensor.matmul(ps[:], lh, rl, start=False, stop=False)
                            nc.tensor.matmul(ps[:], ll, rh, start=False,
                                             stop=(c == DCH - 1))
                        qts = qp_pool.tile([P, BSH], f32, tag="qts")
                        nc.scalar.activation(qts[:], ps[:], AF.Copy, scale=RSCALE)
                        qh_t = qp_pool.tile([P, BSH], bf16, tag="qh_t")
                        nc.vector.tensor_copy(qh_t[:], qts[:])
                        ql_t = qp_pool.tile([P, BSH], bf16, tag="ql_t")
                        nc.vector.scalar_tensor_tensor(ql_t[:], qh_t[:], -1.0, qts[:],
                                                       op0=ALU.mult, op1=ALU.add)
                        nc.sync.dma_start(out=qpart_h[j * P:(j + 1) * P, :], in_=qh_t[:])
                        nc.sync.dma_start(out=qpart_l[j * P:(j + 1) * P, :], in_=ql_t[:])

                nc.gpsimd.collective_compute(
                    "AllGather", ALU.bypass, replica_groups=[list(range(NCORE))],
                    ins=[qpart_h[:]], outs=[qg_h[:]])
                nc.gpsimd.collective_compute(
                    "AllGather", ALU.bypass, replica_groups=[list(range(NCORE))],
                    ins=[qpart_l[:]], outs=[qg_l[:]])

                # ---------------- scores per seed sub-shard ----------------
                for h in range(NHALF):
                    with (
                        tc.tile_pool(name="seedres", bufs=1) as seed_pool,
                        tc.tile_pool(name="sprep", bufs=2) as sprep_pool,
                        tc.tile_pool(name="sps", bufs=2, space="PSUM") as sps_pool,
                    ):
                        shT = seed_pool.tile([P, DCH * SH], bf16, tag="shT")
                        slT = seed_pool.tile([P, DCH * SH], bf16, tag="slT")
                        for t in range(SH // P):
                            srow = sprep_pool.tile([P, D], f32, tag="srow")
                            nc.sync.dma_start(
                                out=srow[:],
                                in_=seeds_sh[h * SH + t * P: h * SH + (t + 1) * P, :])
                            sb = sprep_pool.tile([P, D], bf16, tag="sb")
                            nc.vector.tensor_copy(sb[:], srow[:])
                            sr = sprep_pool.tile([P, D], bf16, tag="sr")
                            nc.vector.scalar_tensor_tensor(sr[:], sb[:], -1.0, srow[:],
                                                           op0=ALU.mult, op1=ALU.add)
                            nc.sync.dma_start(
                                out=sh_dram[h * SH + t * P: h * SH + (t + 1) * P, :],
                                in_=sb[:])
                            for c in range(DCH):
                                ph = sps_pool.tile([P, P], bf16, tag="pst")
                                nc.tensor.transpose(ph[:], sb[:, c * P:(c + 1) * P],
                                                    ident[:])
                                nc.scalar.activation(
                                    shT[:, c * SH + t * P: c * SH + (t + 1) * P],
                                    ph[:], AF.Copy)
                                pl = sps_pool.tile([P, P], bf16, tag="pst")
                                nc.tensor.transpose(pl[:], sr[:, c * P:(c + 1) * P],
                                                    ident[:])
                                nc.scalar.activation(
                                    slT[:, c * SH + t * P: c * SH + (t + 1) * P],
                                    pl[:], AF.Copy)

                        with (
                            tc.tile_pool(name="scr", bufs=2) as scr_pool,
                            tc.tile_pool(name="qrt", bufs=2) as qrt_pool,
                            tc.tile_pool(name="mps", bufs=4, space="PSUM") as mps_pool,
                            tc.tile_pool(name="m8p", bufs=4) as m8_pool,
                        ):
                            for rt in range(RT):
                                qrt_h = qrt_pool.tile([P, DCH * P], bf16, tag="qrt_h")
                                qrt_l = qrt_pool.tile([P, DCH * P], bf16, tag="qrt_l")
                                src_n = rt // (BSH // P)
                                src_o = (rt % (BSH // P)) * P
                                nc.sync.dma_start(
                                    out=qrt_h[:].rearrange("p (c r) -> p c r", c=DCH),
                                    in_=qg_h[src_n, :, src_o:src_o + P]
                                    .rearrange("(c p) r -> p c r", p=P))
                                nc.sync.dma_start(
                                    out=qrt_l[:].rearrange("p (c r) -> p c r", c=DCH),
                                    in_=qg_l[src_n, :, src_o:src_o + P]
                                    .rearrange("(c p) r -> p c r", p=P))

                                blk = scr_pool.tile([P, SH], f32, tag="blk")
                                for st in range(ST):
                                    ps = mps_pool.tile([P, 512], f32, tag="mps")
                                    first = True
                                    for c in range(DCH):
                                        lh = qrt_h[:, c * P:(c + 1) * P]
                                        ll = qrt_l[:, c * P:(c + 1) * P]
                                        rh = shT[:, c * SH + st * 512:
                                                 c * SH + (st + 1) * 512]
                                        rl = slT[:, c * SH + st * 512:
                                                 c * SH + (st + 1) * 512]
                                        nc.tensor.matmul(ps[:], lh, rh, start=first,
                                                         stop=False)
                                        first = False
                                        nc.tensor.matmul(ps[:], lh, rl, start=False,
                                                         stop=False)
                                        nc.tensor.matmul(ps[:], ll, rh, start=False,
                                                         stop=(c == DCH - 1))
                                    nc.vector.scalar_tensor_tensor(
                                        blk[:, st * 512:(st + 1) * 512], ps[:], 1.0,
                                        bias_rep[:, h * SH + st * 512:
                                                 h * SH + (st + 1) * 512],
                                        op0=ALU.mult, op1=ALU.add)
                                nc.sync.dma_start(
                                    out=scores_dram[rt * P:(rt + 1) * P,
                                                    h * SH:(h + 1) * SH],
                                    in_=blk[:])
                                for r in range(4):
                                    m8 = m8_pool.tile([P, 8], f32, tag="m8")
                                    nc.vector.max(out=m8[:], in_=blk[:])
                                    nc.vector.match_replace(out=blk[:], in_to_replace=m8[:],
                                                            in_values=blk[:],
                                                            imm_value=NEG_BIG)
                                    nc.vector.tensor_copy(
                                        loc_all[:, rt * CK + h * K + r * 8:
                                                rt * CK + h * K + (r + 1) * 8], m8[:])

                nc.sync.dma_start(
                    out=cand_dram[:].rearrange("(t p) k -> p t k", p=P),
                    in_=loc_all[:].rearrange("p (t k) -> p t k", t=RT))

            nc.gpsimd.collective_compute(
                "AllGather", ALU.bypass, replica_groups=[list(range(NCORE))],
                ins=[cand_dram[:]], outs=[cand_g[:]])

            # =========== phase B: global merge + attn + field ===========
            with (
                tc.tile_pool(name="shres", bufs=1) as shres_pool,
                tc.tile_pool(name="mrg", bufs=2) as mrg_pool,
                tc.tile_pool(name="wblk", bufs=2) as wblk_pool,
                tc.tile_pool(name="wps", bufs=2, space="PSUM") as wps_pool,
                tc.tile_pool(name="fps", bufs=4, space="PSUM") as fps_pool,
            ):
                sh_res = shres_pool.tile([P, (NSH // P) * D], bf16, tag="sh_res")
                for t in range(NSH // P):
                    nc.sync.dma_start(out=sh_res[:, t * D:(t + 1) * D],
                                      in_=sh_dram[t * P:(t + 1) * P, :])

                for rt in range(RT):
                    mg = mrg_pool.tile([P, NCORE * CK], f32, tag="mg")
                    nc.sync.dma_start(
                        out=mg[:].rearrange("p (n k) -> p n k", n=NCORE),
                        in_=cand_g[:, rt * P:(rt + 1) * P, :]
                        .rearrange("n p k -> p n k"))
                    g32 = mrg_pool.tile([P, K], f32, tag="g32")
                    for r in range(4):
                        m8 = mrg_pool.tile([P, 8], f32, tag="gm8")
                        nc.vector.max(out=m8[:], in_=mg[:])
                        nc.vector.match_replace(out=mg[:], in_to_replace=m8[:],
                                                in_values=mg[:], imm_value=NEG_BIG)
                        nc.vector.tensor_copy(g32[:, r * 8:(r + 1) * 8], m8[:])

                    m0 = mrg_pool.tile([P, 1], f32, tag="m0")
                    nc.vector.tensor_copy(m0[:], g32[:, 0:1])
                    nm0 = mrg_pool.tile([P, 1], f32, tag="nm0")
                    nc.vector.tensor_scalar_mul(nm0[:], m0[:], -1.0)
                    ex = mrg_pool.tile([P, K], f32, tag="ex")
                    dsum = mrg_pool.tile([P, 1], f32, tag="dsum")
                    nc.scalar.activation(ex[:], g32[:], AF.Exp, bias=nm0[:],
                                         accum_out=dsum[:])
                    rec = mrg_pool.tile([P, 1], f32, tag="rec")
                    nc.vector.reciprocal(rec[:], dsum[:])
                    at = mrg_pool.tile([P, K], f32, tag="at")
                    nc.vector.tensor_scalar(at[:], ex[:], rec[:], None, op0=ALU.mult)
                    nc.sync.dma_start(out=attn_out[rt * P:(rt + 1) * P, :], in_=at[:])

                    lnd = mrg_pool.tile([P, 1], f32, tag="lnd")
                    nc.scalar.activation(lnd[:], dsum[:], AF.Ln)
                    b2 = mrg_pool.tile([P, 1], f32, tag="b2")
                    nc.vector.tensor_add(b2[:], lnd[:], m0[:])
                    nc.vector.tensor_scalar_mul(b2[:], b2[:], -1.0)
                    t32 = mrg_pool.tile([P, 1], f32, tag="t32")
                    nc.vector.tensor_copy(t32[:], g32[:, K - 1:K])

                    psf = [fps_pool.tile([P, 512], f32, tag="psf", name=f"psf{rt}_{_i}") for _i in range(FDT)]
                    WH = NSH // 2   # process w in two half-blocks for SBUF
                    for wh_i in range(2):
                        sc = wblk_pool.tile([P, WH], f32, tag="sc")
                        nc.sync.dma_start(
                            out=sc[:],
                            in_=scores_dram[rt * P:(rt + 1) * P,
                                            wh_i * WH:(wh_i + 1) * WH])
                        msk = wblk_pool.tile([P, WH], f32, tag="msk")
                        nc.vector.tensor_scalar(msk[:], sc[:], t32[:], None,
                                                op0=ALU.is_ge)
                        ev = wblk_pool.tile([P, WH], f32, tag="ev")
                        nc.scalar.activation(ev[:], sc[:], AF.Exp, bias=b2[:])
                        wv = wblk_pool.tile([P, WH], bf16, tag="wv")
                        nc.vector.tensor_mul(wv[:], msk[:], ev[:])

                        wt = wblk_pool.tile([P, (WH // P) * P], bf16, tag="wt")
                        for s in range(WH // P):
                            pst = wps_pool.tile([P, P], bf16, tag="wtp")
                            nc.tensor.transpose(pst[:], wv[:, s * P:(s + 1) * P],
                                                ident[:])
                            nc.scalar.activation(wt[:, s * P:(s + 1) * P], pst[:],
                                                 AF.Copy)

                        for dt_i in range(FDT):
                            for s in range(WH // P):
                                sg = wh_i * (WH // P) + s
                                nc.tensor.matmul(
                                    psf[dt_i][:], wt[:, s * P:(s + 1) * P],
                                    sh_res[:, sg * D + dt_i * 512:
                                           sg * D + (dt_i + 1) * 512],
                                    start=(wh_i == 0 and s == 0),
                                    stop=(wh_i == 1 and s == WH // P - 1))

                    for dt_i in range(FDT):
                        fb = wblk_pool.tile([P, 512], f32, tag="fb")
                        nc.scalar.activation(fb[:], psf[dt_i][:], AF.Copy)
                        nc.sync.dma_start(
                            out=fldp_dram[rt * P:(rt + 1) * P,
                                          dt_i * 512:(dt_i + 1) * 512],
                            in_=fb[:])

            nc.gpsimd.collective_compute(
                "ReduceScatter", ALU.add, replica_groups=[list(range(NCORE))],
                ins=[fldp_dram[:]], outs=[rs_out[:]])
            with tc.tile_pool(name="outp", bufs=2) as out_pool:
                for i in range(BSH // P):
                    t = out_pool.tile([P, D], f32, tag="t")
                    nc.sync.dma_start(out=t[:], in_=rs_out[i * P:(i + 1) * P, :])
                    nc.sync.dma_start(out=field_out[i * P:(i + 1) * P, :], in_=t[:])

    nc.compile()
    return nc


_NC_CACHE = None


def kernel(x, W_q, seeds, karma):
    global _NC_CACHE
    x = np.asarray(x, dtype=np.float32)
    W_q = np.asarray(W_q, dtype=np.float32)
    seeds = np.asarray(seeds, dtype=np.float32)
    karma = np.asarray(karma, dtype=np.float32)

    if _NC_CACHE is None:
        _NC_CACHE = build()
    nc = _NC_CACHE

    in_maps = []
    for i in range(NCORE):
        in_maps.append({
            "x_sh": np.ascontiguousarray(x[i * BSH:(i + 1) * BSH]),
            "w_q": W_q,
            "seeds_sh": np.ascontiguousarray(seeds[i * NSH:(i + 1) * NSH]),
            "karma": karma,
            "karma_sh": np.ascontiguousarray(karma[i * NSH:(i + 1) * NSH]),
        })
    import os
    trace = bool(os.environ.get("CHITTA_TRACE"))
    res = run_bass_kernel_spmd(nc, in_maps, list(range(NCORE)), trace=trace)
    if trace and res.exec_time_ns is not None:
        print(f"HW exec time: {res.exec_time_ns} ns", flush=True)
    field = np.concatenate([res.results[i]["field_out"] for i in range(NCORE)], axis=0)
    attn = res.results[0]["attn_out"]
    return field, attn


# revision 7
# speedup vs baseline: 62.0052x; 62.0052x over previous
"""ChittaEncoder retrieval-knn kernel for 8 trn2 NeuronCores.

Tensor-parallel retrieval, seed-sharded (4096 seeds/core):
  - q projection row-sharded, qT all-gathered (bf16 hi/lo split pair).
  - scores for all 2048 rows vs local seed shard via bf16 3-term split
    matmuls (qh*sh + qh*sl + ql*sh) accumulated in fp32 PSUM ->
    fp32-grade score precision at bf16 matmul speed.
  - exact local top-32 per row per seed-quarter via 4 rounds of DVE
    max8 + match_replace; allgather candidates; replicated global
    merge -> sorted global top-32 -> softmax -> attn.
  - field: w = [s >= t32] * exp(s - m - ln denom) bf16 mask-weights;
    partial field = w @ seeds_shard (bf16); ReduceScatter(add).
"""

import numpy as np

import concourse.bacc as bacc
import concourse.mybir as mybir
from concourse.tile import TileContext
from concourse.bass_utils import run_bass_kernel_spmd
from concourse.kernels.tile_matmul import make_identity

P = 128
B, D, N, K = 2048, 2048, 32768, 32
NCORE = 8
NSH = N // NCORE            # 4096 seeds per core
NHALF = 4                   # seed sub-shards per core (SBUF residency)
SH = NSH // NHALF           # 1024 seeds per sub-shard
RT = B // P                 # 16 row tiles
DCH = D // P                # 16 contraction chunks
BSH = B // NCORE            # 256 rows per core for q
ST = SH // 512              # seed tiles of 512 per sub-shard
FDT = D // 512              # d-tiles for the field matmul
CK = NHALF * K              # local candidates per row (128)
RSCALE = 1.0 / float(np.sqrt(np.float32(D)))

f32 = mybir.dt.float32
bf16 = mybir.dt.bfloat16
AF = mybir.ActivationFunctionType
ALU = mybir.AluOpType
AX = mybir.AxisListType

NEG_BIG = -1.0e30


def build(collectives=True):
    import concourse.bass_isa as bass_isa

    nc = bacc.Bacc()

    x_sh = nc.declare_dram_parameter("x_sh", [BSH, D], f32, isOutput=False)
    w_q = nc.declare_dram_parameter("w_q", [D, D], f32, isOutput=False)
    seeds_sh = nc.declare_dram_parameter("seeds_sh", [NSH, D], f32, isOutput=False)
    karma = nc.declare_dram_parameter("karma", [N], f32, isOutput=False)
    karma_sh = nc.declare_dram_parameter("karma_sh", [NSH], f32, isOutput=False)

    attn_out = nc.declare_dram_parameter("attn_out", [B, K], f32, isOutput=True)
    field_out = nc.declare_dram_parameter("field_out", [BSH, D], f32, isOutput=True)

    qpart_h = nc.dram_tensor("qpart_h", [D, BSH], bf16)
    qpart_l = nc.dram_tensor("qpart_l", [D, BSH], bf16)
    qg_h = nc.dram_tensor("qg_h", [NCORE, D, BSH], bf16, addr_space="Shared")
    qg_l = nc.dram_tensor("qg_l", [NCORE, D, BSH], bf16, addr_space="Shared")
    sh_dram = nc.dram_tensor("sh_dram", [NSH, D], bf16)
    scores_dram = nc.dram_tensor("scores_dram", [B, NSH], f32)
    cand_dram = nc.dram_tensor("cand_dram", [B, CK], f32)
    cand_g = nc.dram_tensor("cand_g", [NCORE, B, CK], f32, addr_space="Shared")
    fldp_dram = nc.dram_tensor("fldp_dram", [B, D], f32)
    rs_out = nc.dram_tensor("rs_out", [BSH, D], f32)

    with TileContext(nc) as tc:
        with tc.tile_pool(name="const", bufs=1) as const_pool:
            ident = const_pool.tile([P, P], bf16)
            make_identity(nc, ident)
            ones1 = const_pool.tile([1, P], f32)
            nc.vector.memset(ones1[:], 1.0)

            # =========== phase A: karma bias, q projection, scores, local topk =========
            with tc.tile_pool(name="mid", bufs=1) as mid_pool:
                bias_rep = mid_pool.tile([P, NSH], f32, tag="bias_rep")
                loc_all = mid_pool.tile([P, RT * CK], f32, tag="loc_all")

                # ---------------- karma bias ----------------
                with (
                    tc.tile_pool(name="kb", bufs=1) as kb_pool,
                    tc.tile_pool(name="kbp", bufs=2, space="PSUM") as kbp,
                ):
                    kt = kb_pool.tile([P, N // P], f32)
                    nc.sync.dma_start(out=kt[:], in_=karma[:].rearrange("(p f) -> p f", p=P))
                    kmax = kb_pool.tile([P, 1], f32)
                    nc.vector.tensor_reduce(kmax[:], kt[:], axis=AX.X, op=ALU.max)
                    gmax = kb_pool.tile([P, 1], f32)
                    nc.gpsimd.partition_all_reduce(gmax[:], kmax[:], channels=P,
                                                   reduce_op=bass_isa.ReduceOp.max)
                    ngmax = kb_pool.tile([P, 1], f32)
                    nc.vector.tensor_scalar_mul(ngmax[:], gmax[:], -1.0)
                    esc = kb_pool.tile([P, N // P], f32)
                    ssum = kb_pool.tile([P, 1], f32)
                    nc.scalar.activation(esc[:], kt[:], AF.Exp, bias=ngmax[:],
                                         accum_out=ssum[:])
                    stot = kb_pool.tile([P, 1], f32)
                    nc.gpsimd.partition_all_reduce(stot[:], ssum[:], channels=P,
                                                   reduce_op=bass_isa.ReduceOp.add)
                    lnt = kb_pool.tile([P, 1], f32)
                    nc.scalar.activation(lnt[:], stot[:], AF.Ln)
                    lse = kb_pool.tile([P, 1], f32)
                    nc.vector.tensor_add(lse[:], lnt[:], gmax[:])

                    ksh = kb_pool.tile([1, NSH], f32)
                    nc.sync.dma_start(out=ksh[:],
                                      in_=karma_sh[:].rearrange("(o f) -> o f", o=1))
                    bias_row = kb_pool.tile([1, NSH], f32)
                    nc.vector.tensor_scalar(bias_row[:], ksh[:], lse[0:1, 0:1], None,
                                            op0=ALU.subtract)
                    nc.vector.tensor_scalar_max(bias_row[:], bias_row[:], -10.0)
                    for j in range(NSH // 512):
                        ps = kbp.tile([P, 512], f32, tag="ps")
                        nc.tensor.matmul(ps[:], ones1[:],
                                         bias_row[:, j * 512:(j + 1) * 512],
                                         start=True, stop=True)
                        nc.vector.tensor_copy(bias_rep[:, j * 512:(j + 1) * 512], ps[:])

                # ---------------- q projection ----------------
                with (
                    tc.tile_pool(name="qp", bufs=2) as qp_pool,
                    tc.tile_pool(name="qx", bufs=1) as qx_pool,
                    tc.tile_pool(name="qwj", bufs=2) as qwj_pool,
                    tc.tile_pool(name="qps", bufs=2, space="PSUM") as qps_pool,
                    tc.tile_pool(name="qacc_ps", bufs=2, space="PSUM") as qacc_pool,
                ):
                    xh = qx_pool.tile([P, DCH * BSH], bf16, tag="xh")
                    xl = qx_pool.tile([P, DCH * BSH], bf16, tag="xl")
                    for i in range(BSH // P):
                        xrow = qp_pool.tile([P, D], f32, tag="xrow")
                        nc.sync.dma_start(out=xrow[:], in_=x_sh[i * P:(i + 1) * P, :])
                        xrow_b = qp_pool.tile([P, D], bf16, tag="xrow_b")
                        nc.vector.tensor_copy(xrow_b[:], xrow[:])
                        xrow_r = qp_pool.tile([P, D], bf16, tag="xrow_r")
                        nc.vector.scalar_tensor_tensor(xrow_r[:], xrow_b[:], -1.0, xrow[:],
                                                       op0=ALU.mult, op1=ALU.add)
                        for c in range(DCH):
                            psh = qps_pool.tile([P, P], bf16, tag="pst")
                            nc.tensor.transpose(psh[:], xrow_b[:, c * P:(c + 1) * P],
                                                ident[:])
                            nc.scalar.activation(
                                xh[:, c * BSH + i * P: c * BSH + (i + 1) * P],
                                psh[:], AF.Copy)
                            psl = qps_pool.tile([P, P], bf16, tag="pst")
                            nc.tensor.transpose(psl[:], xrow_r[:, c * P:(c + 1) * P],
                                                ident[:])
                            nc.scalar.activation(
                                xl[:, c * BSH + i * P: c * BSH + (i + 1) * P],
                                psl[:], AF.Copy)

                    for j in range(D // P):
                        wrow = qp_pool.tile([P, D], f32, tag="wrow")
                        nc.sync.dma_start(out=wrow[:], in_=w_q[j * P:(j + 1) * P, :])
                        wrow_b = qp_pool.tile([P, D], bf16, tag="wrow_b")
                        nc.vector.tensor_copy(wrow_b[:], wrow[:])
                        wrow_r = qp_pool.tile([P, D], bf16, tag="wrow_r")
                        nc.vector.scalar_tensor_tensor(wrow_r[:], wrow_b[:], -1.0, wrow[:],
                                                       op0=ALU.mult, op1=ALU.add)
                        wjh = qwj_pool.tile([P, DCH * P], bf16, tag="wjh")
                        wjl = qwj_pool.tile([P, DCH * P], bf16, tag="wjl")
                        for c in range(DCH):
                            psh = qps_pool.tile([P, P], bf16, tag="pst")
                            nc.tensor.transpose(psh[:], wrow_b[:, c * P:(c + 1) * P],
                                                ident[:])
                            nc.scalar.activation(wjh[:, c * P:(c + 1) * P], psh[:], AF.Copy)
                            psl = qps_pool.tile([P, P], bf16, tag="pst")
                            nc.tensor.transpose(psl[:], wrow_r[:, c * P:(c + 1) * P],
                                                ident[:])
                            nc.scalar.activation(wjl[:, c * P:(c + 1) * P], psl[:], AF.Copy)

                        ps = qacc_pool.tile([P, BSH], f32, tag="qacc")
                        first = True
                        for c in range(DCH):
                            lh = wjh[:, c * P:(c + 1) * P]
                            ll = wjl[:, c * P:(c + 1) * P]
                            rh = xh[:, c * BSH:(c + 1) * BSH]
                            rl = xl[:, c * BSH:(c + 1) * BSH]
                            nc.tensor.matmul(ps[:], lh, rh, start=first, stop=False)
                            first = False
                            nc.tensor.matmul(ps[:], lh, rl, start=False, stop=False)
                            nc.tensor.matmul(ps[:], ll, rh, start=False,
                                             stop=(c == DCH - 1))
                        qts = qp_pool.tile([P, BSH], f32, tag="qts")
                        nc.scalar.activation(qts[:], ps[:], AF.Copy, scale=RSCALE)
                        qh_t = qp_pool.tile([P, BSH], bf16, tag="qh_t")
                        nc.vector.tensor_copy(qh_t[:], qts[:])
                        ql_t = qp_pool.tile([P, BSH], bf16, tag="ql_t")
                        nc.vector.scalar_tensor_tensor(ql_t[:], qh_t[:], -1.0, qts[:],
                                                       op0=ALU.mult, op1=ALU.add)
                        nc.sync.dma_start(out=qpart_h[j * P:(j + 1) * P, :], in_=qh_t[:])
                        nc.sync.dma_start(out=qpart_l[j * P:(j + 1) * P, :], in_=ql_t[:])

                if collectives:
                    nc.gpsimd.collective_compute(
                        "AllGather", ALU.bypass, replica_groups=[list(range(NCORE))],
                        ins=[qpart_h[:]], outs=[qg_h[:]])
                    nc.gpsimd.collective_compute(
                        "AllGather", ALU.bypass, replica_groups=[list(range(NCORE))],
                        ins=[qpart_l[:]], outs=[qg_l[:]])
                else:
                    for n in range(NCORE):
                        nc.sync.dma_start(out=qg_h[n], in_=qpart_h[:])
                        nc.sync.dma_start(out=qg_l[n], in_=qpart_l[:])

                # ---------------- scores per seed sub-shard ----------------
                for h in range(NHALF):
                    with (
                        tc.tile_pool(name="seedres", bufs=1) as seed_pool,
                        tc.tile_pool(name="sprep", bufs=2) as sprep_pool,
                        tc.tile_pool(name="sps", bufs=2, space="PSUM") as sps_pool,
                    ):
                        shT = seed_pool.tile([P, DCH * SH], bf16, tag="shT")
                        slT = seed_pool.tile([P, DCH * SH], bf16, tag="slT")
                        for t in range(SH // P):
                            srow = sprep_pool.tile([P, D], f32, tag="srow")
                            nc.sync.dma_start(
                                out=srow[:],
                                in_=seeds_sh[h * SH + t * P: h * SH + (t + 1) * P, :])
                            sb = sprep_pool.tile([P, D], bf16, tag="sb")
                            nc.vector.tensor_copy(sb[:], srow[:])
                            sr = sprep_pool.tile([P, D], bf16, tag="sr")
                            nc.vector.scalar_tensor_tensor(sr[:], sb[:], -1.0, srow[:],
                                                           op0=ALU.mult, op1=ALU.add)
                            nc.sync.dma_start(
                                out=sh_dram[h * SH + t * P: h * SH + (t + 1) * P, :],
                                in_=sb[:])
                            for c in range(DCH):
                                ph = sps_pool.tile([P, P], bf16, tag="pst")
                                nc.tensor.transpose(ph[:], sb[:, c * P:(c + 1) * P],
                                                    ident[:])
                                nc.scalar.activation(
                                    shT[:, c * SH + t * P: c * SH + (t + 1) * P],
                                    ph[:], AF.Copy)
                                pl = sps_pool.tile([P, P], bf16, tag="pst")
                                nc.tensor.transpose(pl[:], sr[:, c * P:(c + 1) * P],
                                                    ident[:])
                                nc.scalar.activation(
                                    slT[:, c * SH + t * P: c * SH + (t + 1) * P],
                                    pl[:], AF.Copy)

                        with (
                            tc.tile_pool(name="scr", bufs=2) as scr_pool,
                            tc.tile_pool(name="qrt", bufs=2) as qrt_pool,
                            tc.tile_pool(name="mps", bufs=4, space="PSUM") as mps_pool,
                            tc.tile_pool(name="m8p", bufs=4) as m8_pool,
                        ):
                            for rt in range(RT):
                                qrt_h = qrt_pool.tile([P, DCH * P], bf16, tag="qrt_h")
                                qrt_l = qrt_pool.tile([P, DCH * P], bf16, tag="qrt_l")
                                src_n = rt // (BSH // P)
                                src_o = (rt % (BSH // P)) * P
                                nc.sync.dma_start(
                                    out=qrt_h[:].rearrange("p (c r) -> p c r", c=DCH),
                                    in_=qg_h[src_n, :, src_o:src_o + P]
                                    .rearrange("(c p) r -> p c r", p=P))
                                nc.sync.dma_start(
                                    out=qrt_l[:].rearrange("p (c r) -> p c r", c=DCH),
                                    in_=qg_l[src_n, :, src_o:src_o + P]
                                    .rearrange("(c p) r -> p c r", p=P))

                                blk = scr_pool.tile([P, SH], f32, tag="blk")
                                for st in range(ST):
                                    ps = mps_pool.tile([P, 512], f32, tag="mps")
                                    first = True
                                    for c in range(DCH):
                                        lh = qrt_h[:, c * P:(c + 1) * P]
                                        ll = qrt_l[:, c * P:(c + 1) * P]
                                        rh = shT[:, c * SH + st * 512:
                                                 c * SH + (st + 1) * 512]
                                        rl = slT[:, c * SH + st * 512:
                                                 c * SH + (st + 1) * 512]
                                        nc.tensor.matmul(ps[:], lh, rh, start=first,
                                                         stop=False)
                                        first = False
                                        nc.tensor.matmul(ps[:], lh, rl, start=False,
                                                         stop=False)
                                        nc.tensor.matmul(ps[:], ll, rh, start=False,
                                                         stop=(c == DCH - 1))
                                    nc.vector.scalar_tensor_tensor(
                                        blk[:, st * 512:(st + 1) * 512], ps[:], 1.0,
                                        bias_rep[:, h * SH + st * 512:
                                                 h * SH + (st + 1) * 512],
                                        op0=ALU.mult, op1=ALU.add)
                                nc.sync.dma_start(
                                    out=scores_dram[rt * P:(rt + 1) * P,
                                                    h * SH:(h + 1) * SH],
                                    in_=blk[:])
                                for r in range(4):
                                    m8 = m8_pool.tile([P, 8], f32, tag="m8")
                                    nc.vector.max(out=m8[:], in_=blk[:])
                                    nc.vector.match_replace(out=blk[:], in_to_replace=m8[:],
                                                            in_values=blk[:],
                                                            imm_value=NEG_BIG)
                                    nc.vector.tensor_copy(
                                        loc_all[:, rt * CK + h * K + r * 8:
                                                rt * CK + h * K + (r + 1) * 8], m8[:])

                nc.sync.dma_start(
                    out=cand_dram[:].rearrange("(t p) k -> p t k", p=P),
                    in_=loc_all[:].rearrange("p (t k) -> p t k", t=RT))

            if collectives:
                nc.gpsimd.collective_compute(
                    "AllGather", ALU.bypass, replica_groups=[list(range(NCORE))],
                    ins=[cand_dram[:]], outs=[cand_g[:]])
            else:
                for n in range(NCORE):
                    nc.sync.dma_start(out=cand_g[n], in_=cand_dram[:])

            # =========== phase B: global merge + attn + field ===========
            with (
                tc.tile_pool(name="shres", bufs=1) as shres_pool,
                tc.tile_pool(name="mrg", bufs=2) as mrg_pool,
                tc.tile_pool(name="wblk", bufs=2) as wblk_pool,
                tc.tile_pool(name="wps", bufs=2, space="PSUM") as wps_pool,
                tc.tile_pool(name="fps", bufs=4, space="PSUM") as fps_pool,
            ):
                sh_res = shres_pool.tile([P, (NSH // P) * D], bf16, tag="sh_res")
                for t in range(NSH // P):
                    nc.sync.dma_start(out=sh_res[:, t * D:(t + 1) * D],
                                      in_=sh_dram[t * P:(t + 1) * P, :])

                for rt in range(RT):
                    mg = mrg_pool.tile([P, NCORE * CK], f32, tag="mg")
                    nc.sync.dma_start(
                        out=mg[:].rearrange("p (n k) -> p n k", n=NCORE),
                        in_=cand_g[:, rt * P:(rt + 1) * P, :]
                        .rearrange("n p k -> p n k"))
                    g32 = mrg_pool.tile([P, K], f32, tag="g32")
                    for r in range(4):
                        m8 = mrg_pool.tile([P, 8], f32, tag="gm8")
                        nc.vector.max(out=m8[:], in_=mg[:])
                        nc.vector.match_replace(out=mg[:], in_to_replace=m8[:],
                                                in_values=mg[:], imm_value=NEG_BIG)
                        nc.vector.tensor_copy(g32[:, r * 8:(r + 1) * 8], m8[:])

                    m0 = mrg_pool.tile([P, 1], f32, tag="m0")
                    nc.vector.tensor_copy(m0[:], g32[:, 0:1])
                    nm0 = mrg_pool.tile([P, 1], f32, tag="nm0")
                    nc.vector.tensor_scalar_mul(nm0[:], m0[:], -1.0)
                    ex = mrg_pool.tile([P, K], f32, tag="ex")
                    dsum = mrg_pool.tile([P, 1], f32, tag="dsum")
                    nc.scalar.activation(ex[:], g32[:], AF.Exp, bias=nm0[:],
                                         accum_out=dsum[:])
                    rec = mrg_pool.tile([P, 1], f32, tag="rec")
                    nc.vector.reciprocal(rec[:], dsum[:])
                    at = mrg_pool.tile([P, K], f32, tag="at")
                    nc.vector.tensor_scalar(at[:], ex[:], rec[:], None, op0=ALU.mult)
                    nc.sync.dma_start(out=attn_out[rt * P:(rt + 1) * P, :], in_=at[:])

                    lnd = mrg_pool.tile([P, 1], f32, tag="lnd")
                    nc.scalar.activation(lnd[:], dsum[:], AF.Ln)
                    b2 = mrg_pool.tile([P, 1], f32, tag="b2")
                    nc.vector.tensor_add(b2[:], lnd[:], m0[:])
                    nc.vector.tensor_scalar_mul(b2[:], b2[:], -1.0)
                    t32 = mrg_pool.tile([P, 1], f32, tag="t32")
                    nc.vector.tensor_copy(t32[:], g32[:, K - 1:K])

                    psf = [fps_pool.tile([P, 512], f32, tag="psf", name=f"psf{rt}_{_i}") for _i in range(FDT)]
                    WH = NSH // 2   # process w in two half-blocks for SBUF
                    for wh_i in range(2):
                        sc = wblk_pool.tile([P, WH], f32, tag="sc")
                        nc.sync.dma_start(
                            out=sc[:],
                            in_=scores_dram[rt * P:(rt + 1) * P,
                                            wh_i * WH:(wh_i + 1) * WH])
                        msk = wblk_pool.tile([P, WH], f32, tag="msk")
                        nc.vector.tensor_scalar(msk[:], sc[:], t32[:], None,
                                                op0=ALU.is_ge)
                        ev = wblk_pool.tile([P, WH], f32, tag="ev")
                        nc.scalar.activation(ev[:], sc[:], AF.Exp, bias=b2[:])
                        wv = wblk_pool.tile([P, WH], bf16, tag="wv")
                        nc.vector.tensor_mul(wv[:], msk[:], ev[:])

                        wt = wblk_pool.tile([P, (WH // P) * P], bf16, tag="wt")
                        for s in range(WH // P):
                            pst = wps_pool.tile([P, P], bf16, tag="wtp")
                            nc.tensor.transpose(pst[:], wv[:, s * P:(s + 1) * P],
                                                ident[:])
                            nc.scalar.activation(wt[:, s * P:(s + 1) * P], pst[:],
                                                 AF.Copy)

                        for dt_i in range(FDT):
                            for s in range(WH // P):
                                sg = wh_i * (WH // P) + s
                                nc.tensor.matmul(
                                    psf[dt_i][:], wt[:, s * P:(s + 1) * P],
                                    sh_res[:, sg * D + dt_i * 512:
                                           sg * D + (dt_i + 1) * 512],
                                    start=(wh_i == 0 and s == 0),
                                    stop=(wh_i == 1 and s == WH // P - 1))

                    for dt_i in range(FDT):
                        fb = wblk_pool.tile([P, 512], f32, tag="fb")
                        nc.scalar.activation(fb[:], psf[dt_i][:], AF.Copy)
                        nc.sync.dma_start(
                            out=fldp_dram[rt * P:(rt + 1) * P,
                                          dt_i * 512:(dt_i + 1) * 512],
                            in_=fb[:])

            if collectives:
                nc.gpsimd.collective_compute(
                    "ReduceScatter", ALU.add, replica_groups=[list(range(NCORE))],
                    ins=[fldp_dram[:]], outs=[rs_out[:]])
            else:
                nc.sync.dma_start(out=rs_out[:], in_=fldp_dram[0:BSH, :])
            with tc.tile_pool(name="outp", bufs=2) as out_pool:
                for i in range(BSH // P):
                    t = out_pool.tile([P, D], f32, tag="t")
                    nc.sync.dma_start(out=t[:], in_=rs_out[i * P:(i + 1) * P, :])
                    nc.sync.dma_start(out=field_out[i * P:(i + 1) * P, :], in_=t[:])

    nc.compile()
    return nc


_NC_CACHE = None


def kernel(x, W_q, seeds, karma):
    global _NC_CACHE
    x = np.asarray(x, dtype=np.float32)
    W_q = np.asarray(W_q, dtype=np.float32)
    seeds = np.asarray(seeds, dtype=np.float32)
    karma = np.asarray(karma, dtype=np.float32)

    if _NC_CACHE is None:
        _NC_CACHE = build()
    nc = _NC_CACHE

    in_maps = []
    for i in range(NCORE):
        in_maps.append({
            "x_sh": np.ascontiguousarray(x[i * BSH:(i + 1) * BSH]),
            "w_q": W_q,
            "seeds_sh": np.ascontiguousarray(seeds[i * NSH:(i + 1) * NSH]),
            "karma": karma,
            "karma_sh": np.ascontiguousarray(karma[i * NSH:(i + 1) * NSH]),
        })
    import os
    trace = bool(os.environ.get("CHITTA_TRACE"))
    res = run_bass_kernel_spmd(nc, in_maps, list(range(NCORE)), trace=trace)
    if trace and res.exec_time_ns is not None:
        print(f"HW exec time: {res.exec_time_ns} ns", flush=True)
    field = np.concatenate([res.results[i]["field_out"] for i in range(NCORE)], axis=0)
    attn = res.results[0]["attn_out"]
    return field, attn


# revision 8
# speedup vs baseline: 66.7077x; 1.0758x over previous
"""ChittaEncoder retrieval-knn kernel for 8 trn2 NeuronCores.

Tensor-parallel retrieval, seed-sharded (4096 seeds/core):
  - q projection row-sharded, qT all-gathered (bf16 hi/lo split pair).
  - scores for all 2048 rows vs local seed shard via bf16 3-term split
    matmuls (qh*sh + qh*sl + ql*sh) accumulated in fp32 PSUM ->
    fp32-grade score precision at bf16 matmul speed.
  - exact local top-32 per row per seed-quarter via 4 rounds of DVE
    max8 + match_replace; allgather candidates; replicated global
    merge -> sorted global top-32 -> softmax -> attn.
  - field: w = [s >= t32] * exp(s - m - ln denom) bf16 mask-weights;
    partial field = w @ seeds_shard (bf16); ReduceScatter(add).
"""

import numpy as np

import concourse.bacc as bacc
import concourse.mybir as mybir
from concourse.tile import TileContext
from concourse.bass_utils import run_bass_kernel_spmd
from concourse.kernels.tile_matmul import make_identity

P = 128
B, D, N, K = 2048, 2048, 32768, 32
NCORE = 8
NSH = N // NCORE            # 4096 seeds per core
NHALF = 4                   # seed sub-shards per core (SBUF residency)
SH = NSH // NHALF           # 1024 seeds per sub-shard
RT = B // P                 # 16 row tiles
DCH = D // P                # 16 contraction chunks
BSH = B // NCORE            # 256 rows per core for q
ST = SH // 512              # seed tiles of 512 per sub-shard
FDT = D // 512              # d-tiles for the field matmul
CK = NHALF * K              # local candidates per row (128)
RSCALE = 1.0 / float(np.sqrt(np.float32(D)))

f32 = mybir.dt.float32
bf16 = mybir.dt.bfloat16
AF = mybir.ActivationFunctionType
ALU = mybir.AluOpType
AX = mybir.AxisListType

NEG_BIG = -1.0e30


def build(collectives=True):
    import concourse.bass_isa as bass_isa

    nc = bacc.Bacc()

    x_sh = nc.declare_dram_parameter("x_sh", [BSH, D], f32, isOutput=False)
    w_q = nc.declare_dram_parameter("w_q", [D, D], f32, isOutput=False)
    seeds_sh = nc.declare_dram_parameter("seeds_sh", [NSH, D], f32, isOutput=False)
    karma = nc.declare_dram_parameter("karma", [N], f32, isOutput=False)
    karma_sh = nc.declare_dram_parameter("karma_sh", [NSH], f32, isOutput=False)

    attn_out = nc.declare_dram_parameter("attn_out", [B, K], f32, isOutput=True)
    field_out = nc.declare_dram_parameter("field_out", [BSH, D], f32, isOutput=True)

    qpart_h = nc.dram_tensor("qpart_h", [D, BSH], bf16)
    qpart_l = nc.dram_tensor("qpart_l", [D, BSH], bf16)
    qg_h = nc.dram_tensor("qg_h", [NCORE, D, BSH], bf16, addr_space="Shared")
    qg_l = nc.dram_tensor("qg_l", [NCORE, D, BSH], bf16, addr_space="Shared")
    sh_dram = nc.dram_tensor("sh_dram", [NSH, D], bf16)
    scores_dram = nc.dram_tensor("scores_dram", [B, NSH], f32)
    cand_dram = nc.dram_tensor("cand_dram", [B, CK], f32)
    cand_g = nc.dram_tensor("cand_g", [NCORE, B, CK], f32, addr_space="Shared")
    fldp_dram = nc.dram_tensor("fldp_dram", [B, D], f32)
    rs_out = nc.dram_tensor("rs_out", [BSH, D], f32)

    with TileContext(nc) as tc:
        with tc.tile_pool(name="const", bufs=1) as const_pool:
            ident = const_pool.tile([P, P], bf16)
            make_identity(nc, ident)
            ones1 = const_pool.tile([1, P], f32)
            nc.vector.memset(ones1[:], 1.0)

            # =========== phase A: karma bias, q projection, scores, local topk =========
            with tc.tile_pool(name="mid", bufs=1) as mid_pool:
                bias_rep = mid_pool.tile([P, NSH], f32, tag="bias_rep")
                loc_all = mid_pool.tile([P, RT * CK], f32, tag="loc_all")

                # ---------------- karma bias ----------------
                with (
                    tc.tile_pool(name="kb", bufs=1) as kb_pool,
                    tc.tile_pool(name="kbp", bufs=2, space="PSUM") as kbp,
                ):
                    kt = kb_pool.tile([P, N // P], f32)
                    nc.sync.dma_start(out=kt[:], in_=karma[:].rearrange("(p f) -> p f", p=P))
                    kmax = kb_pool.tile([P, 1], f32)
                    nc.vector.tensor_reduce(kmax[:], kt[:], axis=AX.X, op=ALU.max)
                    gmax = kb_pool.tile([P, 1], f32)
                    nc.gpsimd.partition_all_reduce(gmax[:], kmax[:], channels=P,
                                                   reduce_op=bass_isa.ReduceOp.max)
                    ngmax = kb_pool.tile([P, 1], f32)
                    nc.vector.tensor_scalar_mul(ngmax[:], gmax[:], -1.0)
                    esc = kb_pool.tile([P, N // P], f32)
                    ssum = kb_pool.tile([P, 1], f32)
                    nc.scalar.activation(esc[:], kt[:], AF.Exp, bias=ngmax[:],
                                         accum_out=ssum[:])
                    stot = kb_pool.tile([P, 1], f32)
                    nc.gpsimd.partition_all_reduce(stot[:], ssum[:], channels=P,
                                                   reduce_op=bass_isa.ReduceOp.add)
                    lnt = kb_pool.tile([P, 1], f32)
                    nc.scalar.activation(lnt[:], stot[:], AF.Ln)
                    lse = kb_pool.tile([P, 1], f32)
                    nc.vector.tensor_add(lse[:], lnt[:], gmax[:])

                    ksh = kb_pool.tile([1, NSH], f32)
                    nc.sync.dma_start(out=ksh[:],
                                      in_=karma_sh[:].rearrange("(o f) -> o f", o=1))
                    bias_row = kb_pool.tile([1, NSH], f32)
                    nc.vector.tensor_scalar(bias_row[:], ksh[:], lse[0:1, 0:1], None,
                                            op0=ALU.subtract)
                    nc.vector.tensor_scalar_max(bias_row[:], bias_row[:], -10.0)
                    for j in range(NSH // 512):
                        ps = kbp.tile([P, 512], f32, tag="ps")
                        nc.tensor.matmul(ps[:], ones1[:],
                                         bias_row[:, j * 512:(j + 1) * 512],
                                         start=True, stop=True)
                        nc.vector.tensor_copy(bias_rep[:, j * 512:(j + 1) * 512], ps[:])

                # ---------------- q projection ----------------
                with (
                    tc.tile_pool(name="qp", bufs=2) as qp_pool,
                    tc.tile_pool(name="qx", bufs=1) as qx_pool,
                    tc.tile_pool(name="qwj", bufs=2) as qwj_pool,
                    tc.tile_pool(name="qps", bufs=2, space="PSUM") as qps_pool,
                    tc.tile_pool(name="qacc_ps", bufs=2, space="PSUM") as qacc_pool,
                ):
                    xh = qx_pool.tile([P, DCH * BSH], bf16, tag="xh")
                    xl = qx_pool.tile([P, DCH * BSH], bf16, tag="xl")
                    for i in range(BSH // P):
                        xrow = qp_pool.tile([P, D], f32, tag="xrow")
                        nc.sync.dma_start(out=xrow[:], in_=x_sh[i * P:(i + 1) * P, :])
                        xrow_b = qp_pool.tile([P, D], bf16, tag="xrow_b")
                        nc.vector.tensor_copy(xrow_b[:], xrow[:])
                        xrow_r = qp_pool.tile([P, D], bf16, tag="xrow_r")
                        nc.vector.scalar_tensor_tensor(xrow_r[:], xrow_b[:], -1.0, xrow[:],
                                                       op0=ALU.mult, op1=ALU.add)
                        for c in range(DCH):
                            psh = qps_pool.tile([P, P], bf16, tag="pst")
                            nc.tensor.transpose(psh[:], xrow_b[:, c * P:(c + 1) * P],
                                                ident[:])
                            nc.scalar.activation(
                                xh[:, c * BSH + i * P: c * BSH + (i + 1) * P],
                                psh[:], AF.Copy)
                            psl = qps_pool.tile([P, P], bf16, tag="pst")
                            nc.tensor.transpose(psl[:], xrow_r[:, c * P:(c + 1) * P],
                                                ident[:])
                            nc.scalar.activation(
                                xl[:, c * BSH + i * P: c * BSH + (i + 1) * P],
                                psl[:], AF.Copy)

                    for j in range(D // P):
                        wrow = qp_pool.tile([P, D], f32, tag="wrow")
                        nc.sync.dma_start(out=wrow[:], in_=w_q[j * P:(j + 1) * P, :])
                        wrow_b = qp_pool.tile([P, D], bf16, tag="wrow_b")
                        nc.vector.tensor_copy(wrow_b[:], wrow[:])
                        wrow_r = qp_pool.tile([P, D], bf16, tag="wrow_r")
                        nc.vector.scalar_tensor_tensor(wrow_r[:], wrow_b[:], -1.0, wrow[:],
                                                       op0=ALU.mult, op1=ALU.add)
                        wjh = qwj_pool.tile([P, DCH * P], bf16, tag="wjh")
                        wjl = qwj_pool.tile([P, DCH * P], bf16, tag="wjl")
                        for c in range(DCH):
                            psh = qps_pool.tile([P, P], bf16, tag="pst")
                            nc.tensor.transpose(psh[:], wrow_b[:, c * P:(c + 1) * P],
                                                ident[:])
                            nc.scalar.activation(wjh[:, c * P:(c + 1) * P], psh[:], AF.Copy)
                            psl = qps_pool.tile([P, P], bf16, tag="pst")
                            nc.tensor.transpose(psl[:], wrow_r[:, c * P:(c + 1) * P],
                                                ident[:])
                            nc.scalar.activation(wjl[:, c * P:(c + 1) * P], psl[:], AF.Copy)

                        ps = qacc_pool.tile([P, BSH], f32, tag="qacc")
                        first = True
                        for c in range(DCH):
                            lh = wjh[:, c * P:(c + 1) * P]
                            ll = wjl[:, c * P:(c + 1) * P]
                            rh = xh[:, c * BSH:(c + 1) * BSH]
                            rl = xl[:, c * BSH:(c + 1) * BSH]
                            nc.tensor.matmul(ps[:], lh, rh, start=first, stop=False)
                            first = False
                            nc.tensor.matmul(ps[:], lh, rl, start=False, stop=False)
                            nc.tensor.matmul(ps[:], ll, rh, start=False,
                                             stop=(c == DCH - 1))
                        qts = qp_pool.tile([P, BSH], f32, tag="qts")
                        nc.scalar.activation(qts[:], ps[:], AF.Copy, scale=RSCALE)
                        qh_t = qp_pool.tile([P, BSH], bf16, tag="qh_t")
                        nc.vector.tensor_copy(qh_t[:], qts[:])
                        ql_t = qp_pool.tile([P, BSH], bf16, tag="ql_t")
                        nc.vector.scalar_tensor_tensor(ql_t[:], qh_t[:], -1.0, qts[:],
                                                       op0=ALU.mult, op1=ALU.add)
                        nc.sync.dma_start(out=qpart_h[j * P:(j + 1) * P, :], in_=qh_t[:])
                        nc.sync.dma_start(out=qpart_l[j * P:(j + 1) * P, :], in_=ql_t[:])

                if collectives:
                    nc.gpsimd.collective_compute(
                        "AllGather", ALU.bypass, replica_groups=[list(range(NCORE))],
                        ins=[qpart_h[:]], outs=[qg_h[:]])
                    nc.gpsimd.collective_compute(
                        "AllGather", ALU.bypass, replica_groups=[list(range(NCORE))],
                        ins=[qpart_l[:]], outs=[qg_l[:]])
                else:
                    for n in range(NCORE):
                        nc.sync.dma_start(out=qg_h[n], in_=qpart_h[:])
                        nc.sync.dma_start(out=qg_l[n], in_=qpart_l[:])

                # ---------------- scores per seed sub-shard ----------------
                for h in range(NHALF):
                    with (
                        tc.tile_pool(name="seedres", bufs=1) as seed_pool,
                        tc.tile_pool(name="sprep", bufs=2) as sprep_pool,
                        tc.tile_pool(name="sps", bufs=2, space="PSUM") as sps_pool,
                    ):
                        shT = seed_pool.tile([P, DCH * SH], bf16, tag="shT")
                        slT = seed_pool.tile([P, DCH * SH], bf16, tag="slT")
                        for t in range(SH // P):
                            srow = sprep_pool.tile([P, D], f32, tag="srow")
                            nc.sync.dma_start(
                                out=srow[:],
                                in_=seeds_sh[h * SH + t * P: h * SH + (t + 1) * P, :])
                            sb = sprep_pool.tile([P, D], bf16, tag="sb")
                            nc.vector.tensor_copy(sb[:], srow[:])
                            sr = sprep_pool.tile([P, D], bf16, tag="sr")
                            nc.vector.scalar_tensor_tensor(sr[:], sb[:], -1.0, srow[:],
                                                           op0=ALU.mult, op1=ALU.add)
                            nc.sync.dma_start(
                                out=sh_dram[h * SH + t * P: h * SH + (t + 1) * P, :],
                                in_=sb[:])
                            for c4 in range(DCH // 4):
                                ph = sps_pool.tile([P, 4 * P], bf16, tag="pst")
                                pl = sps_pool.tile([P, 4 * P], bf16, tag="pst")
                                for u in range(4):
                                    c = c4 * 4 + u
                                    nc.tensor.transpose(ph[:, u * P:(u + 1) * P],
                                                        sb[:, c * P:(c + 1) * P],
                                                        ident[:])
                                    nc.tensor.transpose(pl[:, u * P:(u + 1) * P],
                                                        sr[:, c * P:(c + 1) * P],
                                                        ident[:])
                                nc.scalar.activation(
                                    shT[:].rearrange("p (c s) -> p c s", c=DCH)
                                    [:, c4 * 4:(c4 + 1) * 4, t * P:(t + 1) * P],
                                    ph[:].rearrange("p (u s) -> p u s", u=4), AF.Copy)
                                nc.scalar.activation(
                                    slT[:].rearrange("p (c s) -> p c s", c=DCH)
                                    [:, c4 * 4:(c4 + 1) * 4, t * P:(t + 1) * P],
                                    pl[:].rearrange("p (u s) -> p u s", u=4), AF.Copy)

                        with (
                            tc.tile_pool(name="scr", bufs=2) as scr_pool,
                            tc.tile_pool(name="qrt", bufs=2) as qrt_pool,
                            tc.tile_pool(name="mps", bufs=4, space="PSUM") as mps_pool,
                            tc.tile_pool(name="m8p", bufs=4) as m8_pool,
                        ):
                            for rt in range(RT):
                                qrt_h = qrt_pool.tile([P, DCH * P], bf16, tag="qrt_h")
                                qrt_l = qrt_pool.tile([P, DCH * P], bf16, tag="qrt_l")
                                src_n = rt // (BSH // P)
                                src_o = (rt % (BSH // P)) * P
                                nc.sync.dma_start(
                                    out=qrt_h[:].rearrange("p (c r) -> p c r", c=DCH),
                                    in_=qg_h[src_n, :, src_o:src_o + P]
                                    .rearrange("(c p) r -> p c r", p=P))
                                nc.sync.dma_start(
                                    out=qrt_l[:].rearrange("p (c r) -> p c r", c=DCH),
                                    in_=qg_l[src_n, :, src_o:src_o + P]
                                    .rearrange("(c p) r -> p c r", p=P))

                                blk = scr_pool.tile([P, SH], f32, tag="blk")
                                for st in range(ST):
                                    ps = mps_pool.tile([P, 512], f32, tag="mps")
                                    first = True
                                    for c in range(DCH):
                                        lh = qrt_h[:, c * P:(c + 1) * P]
                                        ll = qrt_l[:, c * P:(c + 1) * P]
                                        rh = shT[:, c * SH + st * 512:
                                                 c * SH + (st + 1) * 512]
                                        rl = slT[:, c * SH + st * 512:
                                                 c * SH + (st + 1) * 512]
                                        nc.tensor.matmul(ps[:], lh, rh, start=first,
                                                         stop=False)
                                        first = False
                                        nc.tensor.matmul(ps[:], lh, rl, start=False,
                                                         stop=False)
                                        nc.tensor.matmul(ps[:], ll, rh, start=False,
                                                         stop=(c == DCH - 1))
                                    nc.vector.scalar_tensor_tensor(
                                        blk[:, st * 512:(st + 1) * 512], ps[:], 1.0,
                                        bias_rep[:, h * SH + st * 512:
                                                 h * SH + (st + 1) * 512],
                                        op0=ALU.mult, op1=ALU.add)
                                nc.sync.dma_start(
                                    out=scores_dram[rt * P:(rt + 1) * P,
                                                    h * SH:(h + 1) * SH],
                                    in_=blk[:])
                                for r in range(4):
                                    m8 = m8_pool.tile([P, 8], f32, tag="m8")
                                    nc.vector.max(out=m8[:], in_=blk[:])
                                    nc.vector.match_replace(out=blk[:], in_to_replace=m8[:],
                                                            in_values=blk[:],
                                                            imm_value=NEG_BIG)
                                    nc.vector.tensor_copy(
                                        loc_all[:, rt * CK + h * K + r * 8:
                                                rt * CK + h * K + (r + 1) * 8], m8[:])

                nc.sync.dma_start(
                    out=cand_dram[:].rearrange("(t p) k -> p t k", p=P),
                    in_=loc_all[:].rearrange("p (t k) -> p t k", t=RT))

            if collectives:
                nc.gpsimd.collective_compute(
                    "AllGather", ALU.bypass, replica_groups=[list(range(NCORE))],
                    ins=[cand_dram[:]], outs=[cand_g[:]])
            else:
                for n in range(NCORE):
                    nc.sync.dma_start(out=cand_g[n], in_=cand_dram[:])

            # =========== phase B: global merge + attn + field ===========
            with (
                tc.tile_pool(name="shres", bufs=1) as shres_pool,
                tc.tile_pool(name="mrg", bufs=2) as mrg_pool,
                tc.tile_pool(name="wblk", bufs=2) as wblk_pool,
                tc.tile_pool(name="wps", bufs=2, space="PSUM") as wps_pool,
                tc.tile_pool(name="fps", bufs=4, space="PSUM") as fps_pool,
            ):
                sh_res = shres_pool.tile([P, (NSH // P) * D], bf16, tag="sh_res")
                for t in range(NSH // P):
                    nc.sync.dma_start(out=sh_res[:, t * D:(t + 1) * D],
                                      in_=sh_dram[t * P:(t + 1) * P, :])

                for rt in range(RT):
                    mg = mrg_pool.tile([P, NCORE * CK], f32, tag="mg")
                    nc.sync.dma_start(
                        out=mg[:].rearrange("p (n k) -> p n k", n=NCORE),
                        in_=cand_g[:, rt * P:(rt + 1) * P, :]
                        .rearrange("n p k -> p n k"))
                    g32 = mrg_pool.tile([P, K], f32, tag="g32")
                    for r in range(4):
                        m8 = mrg_pool.tile([P, 8], f32, tag="gm8")
                        nc.vector.max(out=m8[:], in_=mg[:])
                        nc.vector.match_replace(out=mg[:], in_to_replace=m8[:],
                                                in_values=mg[:], imm_value=NEG_BIG)
                        nc.vector.tensor_copy(g32[:, r * 8:(r + 1) * 8], m8[:])

                    m0 = mrg_pool.tile([P, 1], f32, tag="m0")
                    nc.vector.tensor_copy(m0[:], g32[:, 0:1])
                    nm0 = mrg_pool.tile([P, 1], f32, tag="nm0")
                    nc.vector.tensor_scalar_mul(nm0[:], m0[:], -1.0)
                    ex = mrg_pool.tile([P, K], f32, tag="ex")
                    dsum = mrg_pool.tile([P, 1], f32, tag="dsum")
                    nc.scalar.activation(ex[:], g32[:], AF.Exp, bias=nm0[:],
                                         accum_out=dsum[:])
                    rec = mrg_pool.tile([P, 1], f32, tag="rec")
                    nc.vector.reciprocal(rec[:], dsum[:])
                    at = mrg_pool.tile([P, K], f32, tag="at")
                    nc.vector.tensor_scalar(at[:], ex[:], rec[:], None, op0=ALU.mult)
                    nc.sync.dma_start(out=attn_out[rt * P:(rt + 1) * P, :], in_=at[:])

                    lnd = mrg_pool.tile([P, 1], f32, tag="lnd")
                    nc.scalar.activation(lnd[:], dsum[:], AF.Ln)
                    b2 = mrg_pool.tile([P, 1], f32, tag="b2")
                    nc.vector.tensor_add(b2[:], lnd[:], m0[:])
                    nc.vector.tensor_scalar_mul(b2[:], b2[:], -1.0)
                    t32 = mrg_pool.tile([P, 1], f32, tag="t32")
                    nc.vector.tensor_copy(t32[:], g32[:, K - 1:K])

                    psf = [fps_pool.tile([P, 512], f32, tag="psf", name=f"psf{rt}_{_i}") for _i in range(FDT)]
                    WH = NSH // 2   # process w in two half-blocks for SBUF
                    for wh_i in range(2):
                        sc = wblk_pool.tile([P, WH], f32, tag="sc")
                        nc.sync.dma_start(
                            out=sc[:],
                            in_=scores_dram[rt * P:(rt + 1) * P,
                                            wh_i * WH:(wh_i + 1) * WH])
                        msk = wblk_pool.tile([P, WH], f32, tag="msk")
                        nc.vector.tensor_scalar(msk[:], sc[:], t32[:], None,
                                                op0=ALU.is_ge)
                        ev = wblk_pool.tile([P, WH], f32, tag="ev")
                        nc.scalar.activation(ev[:], sc[:], AF.Exp, bias=b2[:])
                        wv = wblk_pool.tile([P, WH], bf16, tag="wv")
                        nc.vector.tensor_mul(wv[:], msk[:], ev[:])

                        wt = wblk_pool.tile([P, (WH // P) * P], bf16, tag="wt")
                        for s in range(WH // P):
                            pst = wps_pool.tile([P, P], bf16, tag="wtp")
                            nc.tensor.transpose(pst[:], wv[:, s * P:(s + 1) * P],
                                                ident[:])
                            nc.scalar.activation(wt[:, s * P:(s + 1) * P], pst[:],
                                                 AF.Copy)

                        for dt_i in range(FDT):
                            for s in range(WH // P):
                                sg = wh_i * (WH // P) + s
                                nc.tensor.matmul(
                                    psf[dt_i][:], wt[:, s * P:(s + 1) * P],
                                    sh_res[:, sg * D + dt_i * 512:
                                           sg * D + (dt_i + 1) * 512],
                                    start=(wh_i == 0 and s == 0),
                                    stop=(wh_i == 1 and s == WH // P - 1))

                    for dt_i in range(FDT):
                        fb = wblk_pool.tile([P, 512], f32, tag="fb")
                        nc.scalar.activation(fb[:], psf[dt_i][:], AF.Copy)
                        nc.sync.dma_start(
                            out=fldp_dram[rt * P:(rt + 1) * P,
                                          dt_i * 512:(dt_i + 1) * 512],
                            in_=fb[:])

            if collectives:
                nc.gpsimd.collective_compute(
                    "ReduceScatter", ALU.add, replica_groups=[list(range(NCORE))],
                    ins=[fldp_dram[:]], outs=[rs_out[:]])
            else:
                nc.sync.dma_start(out=rs_out[:], in_=fldp_dram[0:BSH, :])
            with tc.tile_pool(name="outp", bufs=2) as out_pool:
                for i in range(BSH // P):
                    t = out_pool.tile([P, D], f32, tag="t")
                    nc.sync.dma_start(out=t[:], in_=rs_out[i * P:(i + 1) * P, :])
                    nc.sync.dma_start(out=field_out[i * P:(i + 1) * P, :], in_=t[:])

    nc.compile()
    return nc


_NC_CACHE = None


def kernel(x, W_q, seeds, karma):
    global _NC_CACHE
    x = np.asarray(x, dtype=np.float32)
    W_q = np.asarray(W_q, dtype=np.float32)
    seeds = np.asarray(seeds, dtype=np.float32)
    karma = np.asarray(karma, dtype=np.float32)

    if _NC_CACHE is None:
        _NC_CACHE = build()
    nc = _NC_CACHE

    in_maps = []
    for i in range(NCORE):
        in_maps.append({
            "x_sh": np.ascontiguousarray(x[i * BSH:(i + 1) * BSH]),
            "w_q": W_q,
            "seeds_sh": np.ascontiguousarray(seeds[i * NSH:(i + 1) * NSH]),
            "karma": karma,
            "karma_sh": np.ascontiguousarray(karma[i * NSH:(i + 1) * NSH]),
        })
    import os
    trace = bool(os.environ.get("CHITTA_TRACE"))
    res = run_bass_kernel_spmd(nc, in_maps, list(range(NCORE)), trace=trace)
    if trace and res.exec_time_ns is not None:
        print(f"HW exec time: {res.exec_time_ns} ns", flush=True)
    field = np.concatenate([res.results[i]["field_out"] for i in range(NCORE)], axis=0)
    attn = res.results[0]["attn_out"]
    return field, attn


# revision 11
# speedup vs baseline: 66.8278x; 1.0018x over previous
"""ChittaEncoder retrieval-knn kernel for 8 trn2 NeuronCores.

Tensor-parallel retrieval, seed-sharded (4096 seeds/core):
  - q projection row-sharded, qT all-gathered (bf16 hi/lo split pair).
  - scores for all 2048 rows vs local seed shard via bf16 3-term split
    matmuls (qh*sh + qh*sl + ql*sh) accumulated in fp32 PSUM ->
    fp32-grade score precision at bf16 matmul speed.
  - exact local top-32 per row per seed-quarter via 4 rounds of DVE
    max8 + match_replace; allgather candidates; replicated global
    merge -> sorted global top-32 -> softmax -> attn.
  - field: w = [s >= t32] * exp(s - m - ln denom) bf16 mask-weights;
    partial field = w @ seeds_shard (bf16); ReduceScatter(add).
"""

import numpy as np

import concourse.bacc as bacc
import concourse.mybir as mybir
from concourse.tile import TileContext
from concourse.bass_utils import run_bass_kernel_spmd
from concourse.kernels.tile_matmul import make_identity

P = 128
B, D, N, K = 2048, 2048, 32768, 32
NCORE = 8
NSH = N // NCORE            # 4096 seeds per core
NHALF = 4                   # seed sub-shards per core (SBUF residency)
SH = NSH // NHALF           # 1024 seeds per sub-shard
RT = B // P                 # 16 row tiles
DCH = D // P                # 16 contraction chunks
BSH = B // NCORE            # 256 rows per core for q
ST = SH // 512              # seed tiles of 512 per sub-shard
FDT = D // 512              # d-tiles for the field matmul
CK = NHALF * K              # local candidates per row (128)
RSCALE = 1.0 / float(np.sqrt(np.float32(D)))

f32 = mybir.dt.float32
bf16 = mybir.dt.bfloat16
AF = mybir.ActivationFunctionType
ALU = mybir.AluOpType
AX = mybir.AxisListType

NEG_BIG = -1.0e30


def build(collectives=True):
    import concourse.bass_isa as bass_isa

    nc = bacc.Bacc()

    x_sh = nc.declare_dram_parameter("x_sh", [BSH, D], f32, isOutput=False)
    w_q = nc.declare_dram_parameter("w_q", [D, D], f32, isOutput=False)
    seeds_sh = nc.declare_dram_parameter("seeds_sh", [NSH, D], f32, isOutput=False)
    karma = nc.declare_dram_parameter("karma", [N], f32, isOutput=False)
    karma_sh = nc.declare_dram_parameter("karma_sh", [NSH], f32, isOutput=False)

    attn_out = nc.declare_dram_parameter("attn_out", [B, K], f32, isOutput=True)
    field_out = nc.declare_dram_parameter("field_out", [BSH, D], f32, isOutput=True)

    qpart_h = nc.dram_tensor("qpart_h", [D, BSH], bf16)
    qpart_l = nc.dram_tensor("qpart_l", [D, BSH], bf16)
    qg_h = nc.dram_tensor("qg_h", [NCORE, D, BSH], bf16, addr_space="Shared")
    qg_l = nc.dram_tensor("qg_l", [NCORE, D, BSH], bf16, addr_space="Shared")
    sh_dram = nc.dram_tensor("sh_dram", [NSH, D], bf16)
    scores_dram = nc.dram_tensor("scores_dram", [B, NSH], f32)
    cand_dram = nc.dram_tensor("cand_dram", [B, CK], f32)
    cand_g = nc.dram_tensor("cand_g", [NCORE, B, CK], f32, addr_space="Shared")
    fldp_dram = nc.dram_tensor("fldp_dram", [B, D], f32)
    rs_out = nc.dram_tensor("rs_out", [BSH, D], f32)

    with TileContext(nc) as tc:
        with tc.tile_pool(name="const", bufs=1) as const_pool:
            ident = const_pool.tile([P, P], bf16)
            make_identity(nc, ident)
            ones1 = const_pool.tile([1, P], f32)
            nc.vector.memset(ones1[:], 1.0)

            # =========== phase A: karma bias, q projection, scores, local topk =========
            with tc.tile_pool(name="mid", bufs=1) as mid_pool:
                bias_rep = mid_pool.tile([P, NSH], f32, tag="bias_rep")
                loc_all = mid_pool.tile([P, RT * CK], f32, tag="loc_all")

                # ---------------- karma bias ----------------
                with (
                    tc.tile_pool(name="kb", bufs=1) as kb_pool,
                    tc.tile_pool(name="kbp", bufs=2, space="PSUM") as kbp,
                ):
                    kt = kb_pool.tile([P, N // P], f32)
                    nc.sync.dma_start(out=kt[:], in_=karma[:].rearrange("(p f) -> p f", p=P))
                    kmax = kb_pool.tile([P, 1], f32)
                    nc.vector.tensor_reduce(kmax[:], kt[:], axis=AX.X, op=ALU.max)
                    gmax = kb_pool.tile([P, 1], f32)
                    nc.gpsimd.partition_all_reduce(gmax[:], kmax[:], channels=P,
                                                   reduce_op=bass_isa.ReduceOp.max)
                    ngmax = kb_pool.tile([P, 1], f32)
                    nc.vector.tensor_scalar_mul(ngmax[:], gmax[:], -1.0)
                    esc = kb_pool.tile([P, N // P], f32)
                    ssum = kb_pool.tile([P, 1], f32)
                    nc.scalar.activation(esc[:], kt[:], AF.Exp, bias=ngmax[:],
                                         accum_out=ssum[:])
                    stot = kb_pool.tile([P, 1], f32)
                    nc.gpsimd.partition_all_reduce(stot[:], ssum[:], channels=P,
                                                   reduce_op=bass_isa.ReduceOp.add)
                    lnt = kb_pool.tile([P, 1], f32)
                    nc.scalar.activation(lnt[:], stot[:], AF.Ln)
                    lse = kb_pool.tile([P, 1], f32)
                    nc.vector.tensor_add(lse[:], lnt[:], gmax[:])

                    ksh = kb_pool.tile([1, NSH], f32)
                    nc.sync.dma_start(out=ksh[:],
                                      in_=karma_sh[:].rearrange("(o f) -> o f", o=1))
                    bias_row = kb_pool.tile([1, NSH], f32)
                    nc.vector.tensor_scalar(bias_row[:], ksh[:], lse[0:1, 0:1], None,
                                            op0=ALU.subtract)
                    nc.vector.tensor_scalar_max(bias_row[:], bias_row[:], -10.0)
                    for j in range(NSH // 512):
                        ps = kbp.tile([P, 512], f32, tag="ps")
                        nc.tensor.matmul(ps[:], ones1[:],
                                         bias_row[:, j * 512:(j + 1) * 512],
                                         start=True, stop=True)
                        nc.vector.tensor_copy(bias_rep[:, j * 512:(j + 1) * 512], ps[:])

                # ---------------- q projection ----------------
                with (
                    tc.tile_pool(name="qp", bufs=2) as qp_pool,
                    tc.tile_pool(name="qx", bufs=1) as qx_pool,
                    tc.tile_pool(name="qwj", bufs=2) as qwj_pool,
                    tc.tile_pool(name="qps", bufs=2, space="PSUM") as qps_pool,
                    tc.tile_pool(name="qacc_ps", bufs=2, space="PSUM") as qacc_pool,
                ):
                    xh = qx_pool.tile([P, DCH * BSH], bf16, tag="xh")
                    xl = qx_pool.tile([P, DCH * BSH], bf16, tag="xl")
                    for i in range(BSH // P):
                        xrow = qp_pool.tile([P, D], f32, tag="xrow")
                        nc.sync.dma_start(out=xrow[:], in_=x_sh[i * P:(i + 1) * P, :])
                        xrow_b = qp_pool.tile([P, D], bf16, tag="xrow_b")
                        nc.vector.tensor_copy(xrow_b[:], xrow[:])
                        xrow_r = qp_pool.tile([P, D], bf16, tag="xrow_r")
                        nc.vector.scalar_tensor_tensor(xrow_r[:], xrow_b[:], -1.0, xrow[:],
                                                       op0=ALU.mult, op1=ALU.add)
                        for c in range(DCH):
                            psh = qps_pool.tile([P, P], bf16, tag="pst")
                            nc.tensor.transpose(psh[:], xrow_b[:, c * P:(c + 1) * P],
                                                ident[:])
                            nc.scalar.activation(
                                xh[:, c * BSH + i * P: c * BSH + (i + 1) * P],
                                psh[:], AF.Copy)
                            psl = qps_pool.tile([P, P], bf16, tag="pst")
                            nc.tensor.transpose(psl[:], xrow_r[:, c * P:(c + 1) * P],
                                                ident[:])
                            nc.scalar.activation(
                                xl[:, c * BSH + i * P: c * BSH + (i + 1) * P],
                                psl[:], AF.Copy)

                    for j in range(D // P):
                        wrow = qp_pool.tile([P, D], f32, tag="wrow")
                        nc.sync.dma_start(out=wrow[:], in_=w_q[j * P:(j + 1) * P, :])
                        wrow_b = qp_pool.tile([P, D], bf16, tag="wrow_b")
                        nc.vector.tensor_copy(wrow_b[:], wrow[:])
                        wrow_r = qp_pool.tile([P, D], bf16, tag="wrow_r")
                        nc.vector.scalar_tensor_tensor(wrow_r[:], wrow_b[:], -1.0, wrow[:],
                                                       op0=ALU.mult, op1=ALU.add)
                        wjh = qwj_pool.tile([P, DCH * P], bf16, tag="wjh")
                        wjl = qwj_pool.tile([P, DCH * P], bf16, tag="wjl")
                        for c in range(DCH):
                            psh = qps_pool.tile([P, P], bf16, tag="pst")
                            nc.tensor.transpose(psh[:], wrow_b[:, c * P:(c + 1) * P],
                                                ident[:])
                            nc.scalar.activation(wjh[:, c * P:(c + 1) * P], psh[:], AF.Copy)
                            psl = qps_pool.tile([P, P], bf16, tag="pst")
                            nc.tensor.transpose(psl[:], wrow_r[:, c * P:(c + 1) * P],
                                                ident[:])
                            nc.scalar.activation(wjl[:, c * P:(c + 1) * P], psl[:], AF.Copy)

                        ps = qacc_pool.tile([P, BSH], f32, tag="qacc")
                        first = True
                        for c in range(DCH):
                            lh = wjh[:, c * P:(c + 1) * P]
                            ll = wjl[:, c * P:(c + 1) * P]
                            rh = xh[:, c * BSH:(c + 1) * BSH]
                            rl = xl[:, c * BSH:(c + 1) * BSH]
                            nc.tensor.matmul(ps[:], lh, rh, start=first, stop=False)
                            first = False
                            nc.tensor.matmul(ps[:], lh, rl, start=False, stop=False)
                            nc.tensor.matmul(ps[:], ll, rh, start=False,
                                             stop=(c == DCH - 1))
                        qts = qp_pool.tile([P, BSH], f32, tag="qts")
                        nc.scalar.activation(qts[:], ps[:], AF.Copy, scale=RSCALE)
                        qh_t = qp_pool.tile([P, BSH], bf16, tag="qh_t")
                        nc.vector.tensor_copy(qh_t[:], qts[:])
                        ql_t = qp_pool.tile([P, BSH], bf16, tag="ql_t")
                        nc.vector.scalar_tensor_tensor(ql_t[:], qh_t[:], -1.0, qts[:],
                                                       op0=ALU.mult, op1=ALU.add)
                        nc.sync.dma_start(out=qpart_h[j * P:(j + 1) * P, :], in_=qh_t[:])
                        nc.sync.dma_start(out=qpart_l[j * P:(j + 1) * P, :], in_=ql_t[:])

                if collectives:
                    nc.gpsimd.collective_compute(
                        "AllGather", ALU.bypass, replica_groups=[list(range(NCORE))],
                        ins=[qpart_h[:]], outs=[qg_h[:]])
                    nc.gpsimd.collective_compute(
                        "AllGather", ALU.bypass, replica_groups=[list(range(NCORE))],
                        ins=[qpart_l[:]], outs=[qg_l[:]])
                else:
                    for n in range(NCORE):
                        nc.sync.dma_start(out=qg_h[n], in_=qpart_h[:])
                        nc.sync.dma_start(out=qg_l[n], in_=qpart_l[:])

                # ---------------- scores per seed sub-shard ----------------
                with (
                    tc.tile_pool(name="seedres", bufs=1) as seed_pool,
                    tc.tile_pool(name="sprep", bufs=3) as sprep_pool,
                    tc.tile_pool(name="sps", bufs=2, space="PSUM") as sps_pool,
                    tc.tile_pool(name="scr", bufs=2) as scr_pool,
                    tc.tile_pool(name="qrt", bufs=2) as qrt_pool,
                    tc.tile_pool(name="mps", bufs=4, space="PSUM") as mps_pool,
                    tc.tile_pool(name="m8p", bufs=4) as m8_pool,
                ):
                  for h in range(NHALF):
                    if True:
                        shT_g = [seed_pool.tile([P, 4 * SH], bf16, tag=f"shT{g}",
                                                 name=f"shT_g{h}_{g}") for g in range(4)]
                        slT_g = [seed_pool.tile([P, 4 * SH], bf16, tag=f"slT{g}",
                                                 name=f"slT_g{h}_{g}") for g in range(4)]
                        for t in range(SH // P):
                            srow = sprep_pool.tile([P, D], f32, tag="srow")
                            nc.sync.dma_start(
                                out=srow[:],
                                in_=seeds_sh[h * SH + t * P: h * SH + (t + 1) * P, :])
                            sb = sprep_pool.tile([P, D], bf16, tag="sb")
                            nc.vector.tensor_copy(sb[:], srow[:])
                            sr = sprep_pool.tile([P, D], bf16, tag="sr")
                            nc.vector.scalar_tensor_tensor(sr[:], sb[:], -1.0, srow[:],
                                                           op0=ALU.mult, op1=ALU.add)
                            nc.sync.dma_start(
                                out=sh_dram[h * SH + t * P: h * SH + (t + 1) * P, :],
                                in_=sb[:])
                            for c4 in range(DCH // 4):
                                ph = sps_pool.tile([P, 4 * P], bf16, tag="pst")
                                pl = sps_pool.tile([P, 4 * P], bf16, tag="pst")
                                for u in range(4):
                                    c = c4 * 4 + u
                                    nc.tensor.transpose(ph[:, u * P:(u + 1) * P],
                                                        sb[:, c * P:(c + 1) * P],
                                                        ident[:])
                                    nc.tensor.transpose(pl[:, u * P:(u + 1) * P],
                                                        sr[:, c * P:(c + 1) * P],
                                                        ident[:])
                                nc.scalar.activation(
                                    shT_g[c4][:].rearrange("p (c s) -> p c s", c=4)
                                    [:, :, t * P:(t + 1) * P],
                                    ph[:].rearrange("p (u s) -> p u s", u=4), AF.Copy)
                                nc.scalar.activation(
                                    slT_g[c4][:].rearrange("p (c s) -> p c s", c=4)
                                    [:, :, t * P:(t + 1) * P],
                                    pl[:].rearrange("p (u s) -> p u s", u=4), AF.Copy)

                        if True:
                            for rt in range(RT):
                                qrt_h = qrt_pool.tile([P, DCH * P], bf16, tag="qrt_h")
                                qrt_l = qrt_pool.tile([P, DCH * P], bf16, tag="qrt_l")
                                src_n = rt // (BSH // P)
                                src_o = (rt % (BSH // P)) * P
                                nc.sync.dma_start(
                                    out=qrt_h[:].rearrange("p (c r) -> p c r", c=DCH),
                                    in_=qg_h[src_n, :, src_o:src_o + P]
                                    .rearrange("(c p) r -> p c r", p=P))
                                nc.sync.dma_start(
                                    out=qrt_l[:].rearrange("p (c r) -> p c r", c=DCH),
                                    in_=qg_l[src_n, :, src_o:src_o + P]
                                    .rearrange("(c p) r -> p c r", p=P))

                                blk = scr_pool.tile([P, SH], f32, tag="blk")
                                for st in range(ST):
                                    ps = mps_pool.tile([P, 512], f32, tag="mps")
                                    first = True
                                    for c in range(DCH):
                                        lh = qrt_h[:, c * P:(c + 1) * P]
                                        ll = qrt_l[:, c * P:(c + 1) * P]
                                        co = (c % 4) * SH
                                        rh = shT_g[c // 4][:, co + st * 512:
                                                           co + (st + 1) * 512]
                                        rl = slT_g[c // 4][:, co + st * 512:
                                                           co + (st + 1) * 512]
                                        nc.tensor.matmul(ps[:], lh, rh, start=first,
                                                         stop=False)
                                        first = False
                                        nc.tensor.matmul(ps[:], lh, rl, start=False,
                                                         stop=False)
                                        nc.tensor.matmul(ps[:], ll, rh, start=False,
                                                         stop=(c == DCH - 1))
                                    nc.vector.scalar_tensor_tensor(
                                        blk[:, st * 512:(st + 1) * 512], ps[:], 1.0,
                                        bias_rep[:, h * SH + st * 512:
                                                 h * SH + (st + 1) * 512],
                                        op0=ALU.mult, op1=ALU.add)
                                nc.sync.dma_start(
                                    out=scores_dram[rt * P:(rt + 1) * P,
                                                    h * SH:(h + 1) * SH],
                                    in_=blk[:])
                                for r in range(4):
                                    m8 = m8_pool.tile([P, 8], f32, tag="m8")
                                    nc.vector.max(out=m8[:], in_=blk[:])
                                    nc.vector.match_replace(out=blk[:], in_to_replace=m8[:],
                                                            in_values=blk[:],
                                                            imm_value=NEG_BIG)
                                    nc.vector.tensor_copy(
                                        loc_all[:, rt * CK + h * K + r * 8:
                                                rt * CK + h * K + (r + 1) * 8], m8[:])

                nc.sync.dma_start(
                    out=cand_dram[:].rearrange("(t p) k -> p t k", p=P),
                    in_=loc_all[:].rearrange("p (t k) -> p t k", t=RT))

            if collectives:
                nc.gpsimd.collective_compute(
                    "AllGather", ALU.bypass, replica_groups=[list(range(NCORE))],
                    ins=[cand_dram[:]], outs=[cand_g[:]])
            else:
                for n in range(NCORE):
                    nc.sync.dma_start(out=cand_g[n], in_=cand_dram[:])

            # =========== phase B: global merge + attn + field ===========
            with (
                tc.tile_pool(name="shres", bufs=1) as shres_pool,
                tc.tile_pool(name="mrg", bufs=2) as mrg_pool,
                tc.tile_pool(name="wblk", bufs=2) as wblk_pool,
                tc.tile_pool(name="wps", bufs=2, space="PSUM") as wps_pool,
                tc.tile_pool(name="fps", bufs=4, space="PSUM") as fps_pool,
            ):
                sh_res = shres_pool.tile([P, (NSH // P) * D], bf16, tag="sh_res")
                for t in range(NSH // P):
                    nc.sync.dma_start(out=sh_res[:, t * D:(t + 1) * D],
                                      in_=sh_dram[t * P:(t + 1) * P, :])

                for rt in range(RT):
                    mg = mrg_pool.tile([P, NCORE * CK], f32, tag="mg")
                    nc.sync.dma_start(
                        out=mg[:].rearrange("p (n k) -> p n k", n=NCORE),
                        in_=cand_g[:, rt * P:(rt + 1) * P, :]
                        .rearrange("n p k -> p n k"))
                    g32 = mrg_pool.tile([P, K], f32, tag="g32")
                    for r in range(4):
                        m8 = mrg_pool.tile([P, 8], f32, tag="gm8")
                        nc.vector.max(out=m8[:], in_=mg[:])
                        nc.vector.match_replace(out=mg[:], in_to_replace=m8[:],
                                                in_values=mg[:], imm_value=NEG_BIG)
                        nc.vector.tensor_copy(g32[:, r * 8:(r + 1) * 8], m8[:])

                    m0 = mrg_pool.tile([P, 1], f32, tag="m0")
                    nc.vector.tensor_copy(m0[:], g32[:, 0:1])
                    nm0 = mrg_pool.tile([P, 1], f32, tag="nm0")
                    nc.vector.tensor_scalar_mul(nm0[:], m0[:], -1.0)
                    ex = mrg_pool.tile([P, K], f32, tag="ex")
                    dsum = mrg_pool.tile([P, 1], f32, tag="dsum")
                    nc.scalar.activation(ex[:], g32[:], AF.Exp, bias=nm0[:],
                                         accum_out=dsum[:])
                    rec = mrg_pool.tile([P, 1], f32, tag="rec")
                    nc.vector.reciprocal(rec[:], dsum[:])
                    at = mrg_pool.tile([P, K], f32, tag="at")
                    nc.vector.tensor_scalar(at[:], ex[:], rec[:], None, op0=ALU.mult)
                    nc.sync.dma_start(out=attn_out[rt * P:(rt + 1) * P, :], in_=at[:])

                    lnd = mrg_pool.tile([P, 1], f32, tag="lnd")
                    nc.scalar.activation(lnd[:], dsum[:], AF.Ln)
                    b2 = mrg_pool.tile([P, 1], f32, tag="b2")
                    nc.vector.tensor_add(b2[:], lnd[:], m0[:])
                    nc.vector.tensor_scalar_mul(b2[:], b2[:], -1.0)
                    t32 = mrg_pool.tile([P, 1], f32, tag="t32")
                    nc.vector.tensor_copy(t32[:], g32[:, K - 1:K])

                    psf = [fps_pool.tile([P, 512], f32, tag="psf", name=f"psf{rt}_{_i}") for _i in range(FDT)]
                    WH = NSH // 2   # process w in two half-blocks for SBUF
                    for wh_i in range(2):
                        sc = wblk_pool.tile([P, WH], f32, tag="sc")
                        nc.sync.dma_start(
                            out=sc[:],
                            in_=scores_dram[rt * P:(rt + 1) * P,
                                            wh_i * WH:(wh_i + 1) * WH])
                        msk = wblk_pool.tile([P, WH], f32, tag="msk")
                        nc.vector.tensor_scalar(msk[:], sc[:], t32[:], None,
                                                op0=ALU.is_ge)
                        ev = wblk_pool.tile([P, WH], f32, tag="ev")
                        nc.scalar.activation(ev[:], sc[:], AF.Exp, bias=b2[:])
                        wv = wblk_pool.tile([P, WH], bf16, tag="wv")
                        nc.vector.tensor_mul(wv[:], msk[:], ev[:])

                        wt = wblk_pool.tile([P, (WH // P) * P], bf16, tag="wt")
                        for s in range(WH // P):
                            pst = wps_pool.tile([P, P], bf16, tag="wtp")
                            nc.tensor.transpose(pst[:], wv[:, s * P:(s + 1) * P],
                                                ident[:])
                            nc.scalar.activation(wt[:, s * P:(s + 1) * P], pst[:],
                                                 AF.Copy)

                        for dt_i in range(FDT):
                            for s in range(WH // P):
                                sg = wh_i * (WH // P) + s
                                nc.tensor.matmul(
                                    psf[dt_i][:], wt[:, s * P:(s + 1) * P],
                                    sh_res[:, sg * D + dt_i * 512:
                                           sg * D + (dt_i + 1) * 512],
                                    start=(wh_i == 0 and s == 0),
                                    stop=(wh_i == 1 and s == WH // P - 1))

                    for dt_i in range(FDT):
                        fb = wblk_pool.tile([P, 512], f32, tag="fb")
                        nc.scalar.activation(fb[:], psf[dt_i][:], AF.Copy)
                        nc.sync.dma_start(
                            out=fldp_dram[rt * P:(rt + 1) * P,
                                          dt_i * 512:(dt_i + 1) * 512],
                            in_=fb[:])

            if collectives:
                nc.gpsimd.collective_compute(
                    "ReduceScatter", ALU.add, replica_groups=[list(range(NCORE))],
                    ins=[fldp_dram[:]], outs=[rs_out[:]])
            else:
                nc.sync.dma_start(out=rs_out[:], in_=fldp_dram[0:BSH, :])
            with tc.tile_pool(name="outp", bufs=2) as out_pool:
                for i in range(BSH // P):
                    t = out_pool.tile([P, D], f32, tag="t")
                    nc.sync.dma_start(out=t[:], in_=rs_out[i * P:(i + 1) * P, :])
                    nc.sync.dma_start(out=field_out[i * P:(i + 1) * P, :], in_=t[:])

    nc.compile()
    return nc


_NC_CACHE = None


def kernel(x, W_q, seeds, karma):
    global _NC_CACHE
    x = np.asarray(x, dtype=np.float32)
    W_q = np.asarray(W_q, dtype=np.float32)
    seeds = np.asarray(seeds, dtype=np.float32)
    karma = np.asarray(karma, dtype=np.float32)

    if _NC_CACHE is None:
        _NC_CACHE = build()
    nc = _NC_CACHE

    in_maps = []
    for i in range(NCORE):
        in_maps.append({
            "x_sh": np.ascontiguousarray(x[i * BSH:(i + 1) * BSH]),
            "w_q": W_q,
            "seeds_sh": np.ascontiguousarray(seeds[i * NSH:(i + 1) * NSH]),
            "karma": karma,
            "karma_sh": np.ascontiguousarray(karma[i * NSH:(i + 1) * NSH]),
        })
    import os
    trace = bool(os.environ.get("CHITTA_TRACE"))
    res = run_bass_kernel_spmd(nc, in_maps, list(range(NCORE)), trace=trace)
    if trace and res.exec_time_ns is not None:
        print(f"HW exec time: {res.exec_time_ns} ns", flush=True)
    field = np.concatenate([res.results[i]["field_out"] for i in range(NCORE)], axis=0)
    attn = res.results[0]["attn_out"]
    return field, attn
